# revision 5
# baseline (speedup 1.0000x reference)
"""Trainium2 Bass kernel for nn_CoherentLoss (histogram_binning).

Math: the coherent-state overlap gt[n] depends on trajectory n only through its
phase-space bin (qb, pb).  With bin centers qc, pc:

  G(qb,pb) = norm * e^{i*pc*qc} * sum_m [wpsi_m * e^{-(x_m-qc)^2}] * e^{-i*pc*x_m}

The m-sum is a matmul  V[qb,m] @ [cos|sin](pc_pb * x_m)[m, pb].  The device
computes V (Gaussian envelopes via ACT Exp) and the cos/sin phase matrices
(ACT Sin after range reduction to [-pi,pi]), and contracts over m on the
TensorEngine.  The m-axis (2401 grid points, padded to 3072 = 8 cores x 3
tiles x 128) is sharded across the 8 NeuronCores; each core emits a partial
[Q, 256] = [Fc | Fs] slab and the host sums the 8 slabs (a 64KB reduction)
and assembles the O(N) tail: binning indices, compact-bin scatter-add, and
the final sum of squares.

ACT Sin is only accurate on [-pi, pi]; reduction uses the VE f32->int32
round-to-nearest conversion:  sin(phi) = Sin(2*pi*(r - rne(r))), r = phi/(2*pi).
"""
from contextlib import ExitStack

import numpy as np

import concourse.bass as bass
from concourse import mybir
from concourse.bass_utils import run_bass_kernel_spmd

QMIN, QMAX, QBINS = -8.0, 8.0, 128
PMIN, PMAX, PBINS = -10.0, 10.0, 128
GAMMA = 1.0
NORM = float((2.0 * GAMMA / np.pi) ** 0.25)
TWO_PI = float(2.0 * np.pi)

N_CORES = 8
f32 = np.float32

_BUILD_CACHE = {}


def _build(T, Q):
    """Build the SPMD program: T m-tiles of 128 per core, Q occupied qb rows."""
    nc = bass.Bass()
    dt = mybir.dt.float32
    CS = 2 * PBINS  # 256: [cos | sin]

    pc_in = nc.declare_dram_parameter("pc", [1, PBINS], dt, isOutput=False)
    qc_in = nc.declare_dram_parameter("qc", [1, Q], dt, isOutput=False)
    xcol_in = nc.declare_dram_parameter("xcol", [128, T], dt, isOutput=False)
    xdiv_in = nc.declare_dram_parameter("xdiv", [128, T], dt, isOutput=False)
    wcol_in = nc.declare_dram_parameter("wcol", [128, T], dt, isOutput=False)
    out = nc.declare_dram_parameter("out", [Q, CS], dt, isOutput=True)

    with ExitStack() as ctx:
        pcb = ctx.enter_context(nc.sbuf_tensor("pcb", [128, PBINS], dt))
        qcb = ctx.enter_context(nc.sbuf_tensor("qcb", [128, Q], dt))
        xcol = ctx.enter_context(nc.sbuf_tensor("xcol_s", [128, T], dt))
        xdiv = ctx.enter_context(nc.sbuf_tensor("xdiv_s", [128, T], dt))
        wcol = ctx.enter_context(nc.sbuf_tensor("wcol_s", [128, T], dt))
        cs = [ctx.enter_context(nc.sbuf_tensor(f"cs{t}", [128, CS], dt)) for t in range(T)]
        ki = [ctx.enter_context(nc.sbuf_tensor(f"ki{t}", [128, CS], mybir.dt.int32)) for t in range(T)]
        kf = [ctx.enter_context(nc.sbuf_tensor(f"kf{t}", [128, CS], dt)) for t in range(T)]
        red = [ctx.enter_context(nc.sbuf_tensor(f"red{t}", [128, CS], dt)) for t in range(T)]
        dd = [ctx.enter_context(nc.sbuf_tensor(f"dd{t}", [128, Q], dt)) for t in range(T)]
        ee = [ctx.enter_context(nc.sbuf_tensor(f"ee{t}", [128, Q], dt)) for t in range(T)]
        vv = [ctx.enter_context(nc.sbuf_tensor(f"vv{t}", [128, Q], dt)) for t in range(T)]
        outs = ctx.enter_context(nc.sbuf_tensor("outs", [Q, CS], dt))
        ps = ctx.enter_context(nc.psum_tensor("ps", [Q, CS], dt))
        dsem = ctx.enter_context(nc.semaphore("dsem"))
        vsem = ctx.enter_context(nc.semaphore("vsem"))
        asem = ctx.enter_context(nc.semaphore("asem"))
        msem = ctx.enter_context(nc.semaphore("msem"))
        csem = ctx.enter_context(nc.semaphore("csem"))
        gsem = ctx.enter_context(nc.semaphore("gsem"))
        blk = nc.Block()
        block = blk.__enter__()

        @block.sync
        def _(sync):
            sync.dma_start(out=pcb[:, :], in_=pc_in[:, :].to_broadcast([128, PBINS])).then_inc(dsem, 16)
            sync.dma_start(out=qcb[:, :], in_=qc_in[:, :].to_broadcast([128, Q])).then_inc(dsem, 16)
            sync.dma_start(out=xcol[:, :], in_=xcol_in[:, :]).then_inc(dsem, 16)
            sync.dma_start(out=xdiv[:, :], in_=xdiv_in[:, :]).then_inc(dsem, 16)
            sync.dma_start(out=wcol[:, :], in_=wcol_in[:, :]).then_inc(dsem, 16)

        @block.vector
        def _(vector):
            vector.wait_ge(dsem, 80)
            for t in range(T):
                # Gaussian envelope argument: d = qc - x_p (sign irrelevant)
                vector.tensor_scalar_sub(dd[t][:, :], qcb[:, :], xcol[:, t : t + 1])
                vector.tensor_mul(dd[t][:, :], dd[t][:, :], dd[t][:, :]).then_inc(vsem, 1)
                # phase/2pi: right half = sin arg, left half = cos arg (+1/4 turn)
                vector.tensor_scalar_mul(cs[t][:, PBINS:], pcb[:, :], xdiv[:, t : t + 1])
                vector.tensor_scalar_add(cs[t][:, :PBINS], cs[t][:, PBINS:], 0.25)
                vector.tensor_copy(ki[t][:, :], cs[t][:, :])   # f32 -> i32 (rne)
                vector.tensor_copy(kf[t][:, :], ki[t][:, :])   # i32 -> f32
                vector.tensor_sub(red[t][:, :], cs[t][:, :], kf[t][:, :]).then_inc(vsem, 1)
                # V = wpsi * exp(-d2): needs ACT Exp of this tile
                vector.wait_ge(asem, 2 * t + 1)
                vector.tensor_scalar_mul(vv[t][:, :], ee[t][:, :], wcol[:, t : t + 1]).then_inc(vsem, 1)

        @block.scalar
        def _(scalar):
            for t in range(T):
                scalar.wait_ge(vsem, 3 * t + 1)
                scalar.activation(ee[t][:, :], dd[t][:, :], mybir.ActivationFunctionType.Exp, scale=-1.0).then_inc(asem, 1)
                scalar.wait_ge(vsem, 3 * t + 2)
                scalar.activation(cs[t][:, :], red[t][:, :], mybir.ActivationFunctionType.Sin, scale=TWO_PI).then_inc(asem, 1)
            # copy accumulated PSUM to SBUF once matmuls are done
            scalar.wait_ge(msem, 1)
            scalar.copy(outs[:, :], ps[:, :]).then_inc(csem, 1)

        @block.tensor
        def _(tensor):
            for t in range(T):
                tensor.wait_ge(vsem, 3 * t + 3)   # V_t ready
                tensor.wait_ge(asem, 2 * t + 2)   # Sin(cs_t) ready
                mm = tensor.matmul(ps[:, :], lhsT=vv[t][:, :], rhs=cs[t][:, :],
                                   start=(t == 0), stop=(t == T - 1))
                if t == T - 1:
                    mm.then_inc(msem, 1)

        @block.gpsimd
        def _(gpsimd):
            gpsimd.wait_ge(csem, 1)
            gpsimd.dma_start(out=out[:, :], in_=outs[:, :]).then_inc(gsem, 16)
            gpsimd.wait_ge(gsem, 16)

        blk.__exit__(None, None, None)

    # clear sem/DMA state so the NEFF is re-runnable (profiling reruns it)
    nc.reset()
    return nc


def _host_prep(q_re, q_im, p_re, p_im, x, psi):
    qf = q_re - p_im / f32(2.0)
    pf = f32(2.0) * q_im + p_re
    dq = f32((QMAX - QMIN) / QBINS)
    dp = f32((PMAX - PMIN) / PBINS)
    qb = np.floor((qf - f32(QMIN)) / dq)
    pb = np.floor((pf - f32(PMIN)) / dp)
    bins = (qb * PBINS + pb).astype(np.int32).reshape(-1)
    uniq, inv = np.unique(bins, return_inverse=True)
    qbi = qb.astype(np.int64).reshape(-1)
    pbi = pb.astype(np.int64).reshape(-1)
    qb_occ = np.unique(qbi)
    qb_rank = {v: i for i, v in enumerate(qb_occ)}
    qb_row = np.array([qb_rank[v] for v in qbi], dtype=np.int64)
    qc_occ = (qb_occ.astype(f32) + f32(0.5)) * dq + f32(QMIN)
    pc_all = (np.arange(PBINS, dtype=f32) + f32(0.5)) * dp + f32(PMIN)
    dx = np.diff(x)
    w = np.zeros_like(x)
    w[0] = dx[0] / 2
    w[-1] = dx[-1] / 2
    w[1:-1] = (dx[:-1] + dx[1:]) / 2
    wpsi = (w * psi).astype(f32)
    return bins, uniq, inv, qb_row, pbi, qc_occ, pc_all, wpsi


def _run_device(x, wpsi, qc_occ, pc_all, trace=False):
    M = x.shape[0]
    Qocc = qc_occ.shape[0]
    Q = max(8, int(np.ceil(Qocc / 8.0)) * 8)
    assert Q <= 128, "qb occupancy exceeds one PSUM partition tile"
    T = int(np.ceil(M / (N_CORES * 128.0)))
    Mp = N_CORES * T * 128

    xp = np.full(Mp, 50.0, dtype=f32)        # pad far from any qc -> env = 0
    wp = np.zeros(Mp, dtype=f32)
    xp[:M] = x
    wp[:M] = wpsi
    qc_pad = np.full(Q, 1000.0, dtype=f32)   # pad columns -> V = 0
    qc_pad[:Qocc] = qc_occ

    xs = xp.reshape(N_CORES, T, 128)
    ws = wp.reshape(N_CORES, T, 128)
    pc_row = np.ascontiguousarray(pc_all.reshape(1, PBINS))
    qc_row = np.ascontiguousarray(qc_pad.reshape(1, Q))

    key = (T, Q)
    if key not in _BUILD_CACHE:
        _BUILD_CACHE[key] = _build(T, Q)
    nc = _BUILD_CACHE[key]

    in_maps = []
    for c in range(N_CORES):
        in_maps.append({
            "pc": pc_row,
            "qc": qc_row,
            "xcol": np.ascontiguousarray(xs[c].T),
            "xdiv": np.ascontiguousarray((xs[c].T / f32(TWO_PI)).astype(f32)),
            "wcol": np.ascontiguousarray(ws[c].T),
        })

    res = run_bass_kernel_spmd(nc, in_maps, core_ids=list(range(N_CORES)), trace=trace)
    F = np.zeros((Q, 2 * PBINS), dtype=np.float64)
    for c in range(N_CORES):
        F += res.results[c]["out"]
    F = F.astype(f32)
    return F[:Qocc, :PBINS], F[:Qocc, PBINS:], res


def kernel(factors_re, factors_im, q_re, q_im, p_re, p_im, x, psi):
    factors_re = np.asarray(factors_re, dtype=f32)
    factors_im = np.asarray(factors_im, dtype=f32)
    q_re = np.asarray(q_re, dtype=f32)
    q_im = np.asarray(q_im, dtype=f32)
    p_re = np.asarray(p_re, dtype=f32)
    p_im = np.asarray(p_im, dtype=f32)
    x = np.asarray(x, dtype=f32)
    psi = np.asarray(psi, dtype=f32)

    bins, uniq, inv, qb_row, pbi, qc_occ, pc_all, wpsi = _host_prep(
        q_re, q_im, p_re, p_im, x, psi
    )
    Fc, Fs, _ = _run_device(x, wpsi, qc_occ, pc_all)

    # ---- host tail: phase correction, gather, scatter-add, loss ----
    phi = (qc_occ[:, None] * pc_all[None, :]).astype(f32)
    cphi = np.cos(phi, dtype=f32)
    sphi = np.sin(phi, dtype=f32)
    G_re = f32(NORM) * (cphi * Fc + sphi * Fs)
    G_im = f32(NORM) * (sphi * Fc - cphi * Fs)
    gt_re = G_re[qb_row, pbi]
    gt_im = G_im[qb_row, pbi]

    e = np.exp((q_im * q_im).astype(f32), dtype=f32)
    ang = (p_re * q_im).astype(f32)
    pr = np.clip(np.nan_to_num(f32(NORM) * e * np.cos(ang, dtype=f32)), -100.0, 100.0).astype(f32)
    pi_ = np.clip(np.nan_to_num(f32(NORM) * e * np.sin(ang, dtype=f32)), -100.0, 100.0).astype(f32)
    vr = (pr * factors_re - pi_ * factors_im).astype(f32).reshape(-1)
    vi = (pr * factors_im + pi_ * factors_re).astype(f32).reshape(-1)

    N = vr.size
    B_re = np.zeros(N, dtype=f32)
    B_im = np.zeros(N, dtype=f32)
    np.add.at(B_re, inv, vr)
    np.add.at(B_im, inv, vi)
    dr = B_re - gt_re
    di = B_im - gt_im
    loss = np.sum(dr * dr + di * di, dtype=f32)
    return np.sqrt(loss, dtype=f32)


# revision 7
# speedup vs baseline: 1.1276x; 1.1276x over previous
"""Trainium2 Bass kernel for nn_CoherentLoss (histogram_binning).

Math: the coherent-state overlap gt[n] depends on trajectory n only through its
phase-space bin (qb, pb).  With bin centers qc, pc:

  G(qb,pb) = norm * e^{i*pc*qc} * sum_m [wpsi_m * e^{-(x_m-qc)^2}] * e^{-i*pc*x_m}

The m-sum is a matmul  V[qb,m] @ [cos|sin](pc_pb * x_m)[m, pb].  The m-axis
(2401 grid points, padded to 3072 = 8 cores x 3 tiles x 128) is sharded across
the 8 NeuronCores; each core emits a partial [Q, 256] = [Fc | Fs] slab and the
host sums the 8 slabs (a 64KB reduction) and assembles the O(N) tail: binning
indices, compact-bin scatter-add, and the final sum of squares.

Device structure per core (m within a tile is affine in the partition index p:
x[p,t] = a_t + h*p, so index-structured matrices are low-rank in p x (t,bin)
and are generated on the TensorEngine from tiny coefficient rows):

  ph[p, (t,j)] = pc_j*x[p,t]/2pi          K=2 matmul  [1,p] x coeffs
  dsq[p, (t,q)] = (qc_q - x[p,t])^2       K=3 matmul  [1,p,p^2] x coeffs
  ee = Exp(-dsq)               one ACT instruction (one table load)
  red = ph - rne(ph)           VE f32->i32->f32 round-trip (range reduction)
  cs = Sin(2pi*red)            one ACT instruction; [cos|sin] via +1/4 turn
  vv = wpsi * ee               VE per-tile scalar mul
  F += vv_t^T @ cs_t           K=128 accumulating matmuls -> PSUM [Q,256]

ACT Sin is only accurate on [-pi, pi], hence the explicit range reduction.
The tail clears semaphores (no barrier) so the NEFF is re-runnable under
profiling.
"""
from contextlib import ExitStack

import numpy as np

import concourse.bass as bass
from concourse import mybir
from concourse.bass_utils import run_bass_kernel_spmd

QMIN, QMAX, QBINS = -8.0, 8.0, 128
PMIN, PMAX, PBINS = -10.0, 10.0, 128
GAMMA = 1.0
NORM = float((2.0 * GAMMA / np.pi) ** 0.25)
TWO_PI = float(2.0 * np.pi)

N_CORES = 8
f32 = np.float32

_BUILD_CACHE = {}


def _build(T, Q):
    """Build the SPMD program: T m-tiles of 128 per core, Q occupied qb rows."""
    nc = bass.Bass()
    dt = mybir.dt.float32
    CS = 2 * PBINS            # 256: [cos | sin] per tile
    WCS = T * CS              # 768
    WQ = T * Q                # 192
    WR = WCS + WQ             # 960: rhs coefficient columns

    lhs_in = nc.declare_dram_parameter("lhs", [3, 128], dt, isOutput=False)
    rhs_in = nc.declare_dram_parameter("rhs", [3, WR], dt, isOutput=False)
    wcol_in = nc.declare_dram_parameter("wcol", [128, T], dt, isOutput=False)
    out = nc.declare_dram_parameter("out", [Q, CS], dt, isOutput=True)

    with ExitStack() as ctx:
        lhs = ctx.enter_context(nc.sbuf_tensor("lhs_s", [3, 128], dt))
        rhs = ctx.enter_context(nc.sbuf_tensor("rhs_s", [3, WR], dt))
        wcol = ctx.enter_context(nc.sbuf_tensor("wcol_s", [128, T], dt))
        ki = ctx.enter_context(nc.sbuf_tensor("ki", [128, WCS], mybir.dt.int32))
        kf = ctx.enter_context(nc.sbuf_tensor("kf", [128, WCS], dt))
        red = ctx.enter_context(nc.sbuf_tensor("red", [128, WCS], dt))
        cs = ctx.enter_context(nc.sbuf_tensor("cs", [128, WCS], dt))
        ee = ctx.enter_context(nc.sbuf_tensor("ee", [128, WQ], dt))
        vv = ctx.enter_context(nc.sbuf_tensor("vv", [128, WQ], dt))
        outs = ctx.enter_context(nc.sbuf_tensor("outs", [Q, CS], dt))
        ph = ctx.enter_context(nc.psum_tensor("ph", [128, WCS], dt))
        dq = ctx.enter_context(nc.psum_tensor("dq", [128, WQ], dt))
        ps = ctx.enter_context(nc.psum_tensor("ps", [Q, CS], dt))
        d1 = ctx.enter_context(nc.semaphore("d1"))
        p1 = ctx.enter_context(nc.semaphore("p1"))
        v1 = ctx.enter_context(nc.semaphore("v1"))
        a1 = ctx.enter_context(nc.semaphore("a1"))
        m1 = ctx.enter_context(nc.semaphore("m1"))
        c1 = ctx.enter_context(nc.semaphore("c1"))
        g1 = ctx.enter_context(nc.semaphore("g1"))
        blk = nc.Block()
        block = blk.__enter__()

        @block.sync
        def _(sync):
            sync.dma_start(out=lhs[:, :], in_=lhs_in[:, :]).then_inc(d1, 16)
            sync.dma_start(out=rhs[:, :], in_=rhs_in[:, :]).then_inc(d1, 16)
            sync.dma_start(out=wcol[:, :], in_=wcol_in[:, :]).then_inc(d1, 16)

        @block.tensor
        def _(tensor):
            tensor.wait_ge(d1, 48)
            # phase/2pi: K=2, split 512 + 256 (PSUM bank limit)
            tensor.matmul(ph[:, 0:512], lhsT=lhs[0:2, :], rhs=rhs[0:2, 0:512],
                          start=True, stop=True)
            tensor.matmul(ph[:, 512:WCS], lhsT=lhs[0:2, :], rhs=rhs[0:2, 512:WCS],
                          start=True, stop=True).then_inc(p1, 1)
            # squared Gaussian argument: K=3
            tensor.matmul(dq[:, :], lhsT=lhs[0:3, :], rhs=rhs[0:3, WCS:WR],
                          start=True, stop=True).then_inc(p1, 1)
            # accumulating contraction over m
            tensor.wait_ge(v1, 2)
            tensor.wait_ge(a1, 2)
            mm = None
            for t in range(T):
                mm = tensor.matmul(ps[:, :], lhsT=vv[:, t * Q:(t + 1) * Q],
                                   rhs=cs[:, t * CS:(t + 1) * CS],
                                   start=(t == 0), stop=(t == T - 1))
            mm.then_inc(m1, 1)

        @block.vector
        def _(vector):
            vector.wait_ge(p1, 1)
            vector.tensor_copy(ki[:, :], ph[:, :])    # f32 -> i32 (round nearest)
            vector.tensor_copy(kf[:, :], ki[:, :])    # i32 -> f32
            vector.tensor_sub(red[:, :], ph[:, :], kf[:, :]).then_inc(v1, 1)
            # V = wpsi * exp(-d^2)
            vector.wait_ge(a1, 1)
            mm = None
            for t in range(T):
                mm = vector.tensor_scalar_mul(vv[:, t * Q:(t + 1) * Q],
                                              ee[:, t * Q:(t + 1) * Q],
                                              wcol[:, t:t + 1])
            mm.then_inc(v1, 1)

        @block.scalar
        def _(scalar):
            scalar.wait_ge(p1, 2)
            scalar.activation(ee[:, :], dq[:, :], mybir.ActivationFunctionType.Exp,
                              scale=-1.0).then_inc(a1, 1)
            scalar.wait_ge(v1, 1)
            scalar.activation(cs[:, :], red[:, :], mybir.ActivationFunctionType.Sin,
                              scale=TWO_PI).then_inc(a1, 1)
            scalar.wait_ge(m1, 1)
            scalar.copy(outs[:, :], ps[:, :]).then_inc(c1, 1)

        @block.gpsimd
        def _(gpsimd):
            gpsimd.wait_ge(c1, 1)
            gpsimd.dma_start(out=out[:, :], in_=outs[:, :]).then_inc(g1, 16)
            gpsimd.wait_ge(g1, 16)

        blk.__exit__(None, None, None)

    nc.reset()  # clear sem/DMA state so the NEFF is re-runnable
    return nc


def _host_prep(q_re, q_im, p_re, p_im, x, psi):
    qf = q_re - p_im / f32(2.0)
    pf = f32(2.0) * q_im + p_re
    dq = f32((QMAX - QMIN) / QBINS)
    dp = f32((PMAX - PMIN) / PBINS)
    qb = np.floor((qf - f32(QMIN)) / dq)
    pb = np.floor((pf - f32(PMIN)) / dp)
    bins = (qb * PBINS + pb).astype(np.int32).reshape(-1)
    uniq, inv = np.unique(bins, return_inverse=True)
    qbi = qb.astype(np.int64).reshape(-1)
    pbi = pb.astype(np.int64).reshape(-1)
    qb_occ = np.unique(qbi)
    qb_rank = {v: i for i, v in enumerate(qb_occ)}
    qb_row = np.array([qb_rank[v] for v in qbi], dtype=np.int64)
    qc_occ = (qb_occ.astype(f32) + f32(0.5)) * dq + f32(QMIN)
    pc_all = (np.arange(PBINS, dtype=f32) + f32(0.5)) * dp + f32(PMIN)
    dx = np.diff(x)
    w = np.zeros_like(x)
    w[0] = dx[0] / 2
    w[-1] = dx[-1] / 2
    w[1:-1] = (dx[:-1] + dx[1:]) / 2
    wpsi = (w * psi).astype(f32)
    return bins, uniq, inv, qb_row, pbi, qc_occ, pc_all, wpsi


def _run_device(x, wpsi, qc_occ, pc_all, trace=False):
    M = x.shape[0]
    Qocc = qc_occ.shape[0]
    Q = max(8, int(np.ceil(Qocc / 8.0)) * 8)
    assert Q <= 128, "qb occupancy exceeds one PSUM partition tile"
    T = int(np.ceil(M / (N_CORES * 128.0)))
    Mp = N_CORES * T * 128
    CS = 2 * PBINS

    # grid is uniform: x[m] = x0 + m*h
    h = f32((float(x[-1]) - float(x[0])) / (M - 1))
    x0 = f32(x[0])

    wp = np.zeros(Mp, dtype=f32)
    wp[:M] = wpsi
    qc_pad = np.full(Q, 1000.0, dtype=f32)   # pad columns -> V = 0
    qc_pad[:Qocc] = qc_occ

    ws = wp.reshape(N_CORES, T, 128)

    key = (T, Q)
    if key not in _BUILD_CACHE:
        _BUILD_CACHE[key] = _build(T, Q)
    nc = _BUILD_CACHE[key]

    # lhsT rows: [ones, p, p^2] (same on all cores)
    p_idx = np.arange(128, dtype=f32)
    lhs = np.ascontiguousarray(np.stack([np.ones(128, f32), p_idx, p_idx * p_idx]).astype(f32))

    in_maps = []
    for c in range(N_CORES):
        # x[p, t] = a_t + h*p for this core's tiles
        a_t = (x0 + h * (np.arange(T, dtype=f32) + f32(c * T)) * f32(128.0)).astype(f32)
        rhs = np.zeros((3, T * CS + T * Q), dtype=f32)
        r1 = (pc_all * h / f32(TWO_PI)).astype(f32)
        for t in range(T):
            base = t * CS
            r0 = (pc_all * a_t[t] / f32(TWO_PI)).astype(f32)
            rhs[0, base:base + PBINS] = r0 + f32(0.25)   # cos args (quarter turn)
            rhs[1, base:base + PBINS] = r1
            rhs[0, base + PBINS:base + CS] = r0          # sin args
            rhs[1, base + PBINS:base + CS] = r1
            qbase = T * CS + t * Q
            dqa = (qc_pad - a_t[t]).astype(f32)
            rhs[0, qbase:qbase + Q] = dqa * dqa
            rhs[1, qbase:qbase + Q] = f32(-2.0) * h * dqa
            rhs[2, qbase:qbase + Q] = h * h
        in_maps.append({
            "lhs": lhs,
            "rhs": rhs,
            "wcol": np.ascontiguousarray(ws[c].T),
        })

    res = run_bass_kernel_spmd(nc, in_maps, core_ids=list(range(N_CORES)), trace=trace)
    F = np.zeros((Q, CS), dtype=np.float64)
    for c in range(N_CORES):
        F += res.results[c]["out"]
    F = F.astype(f32)
    return F[:Qocc, :PBINS], F[:Qocc, PBINS:], res


def kernel(factors_re, factors_im, q_re, q_im, p_re, p_im, x, psi):
    factors_re = np.asarray(factors_re, dtype=f32)
    factors_im = np.asarray(factors_im, dtype=f32)
    q_re = np.asarray(q_re, dtype=f32)
    q_im = np.asarray(q_im, dtype=f32)
    p_re = np.asarray(p_re, dtype=f32)
    p_im = np.asarray(p_im, dtype=f32)
    x = np.asarray(x, dtype=f32)
    psi = np.asarray(psi, dtype=f32)

    bins, uniq, inv, qb_row, pbi, qc_occ, pc_all, wpsi = _host_prep(
        q_re, q_im, p_re, p_im, x, psi
    )
    Fc, Fs, _ = _run_device(x, wpsi, qc_occ, pc_all)

    # ---- host tail: phase correction, gather, scatter-add, loss ----
    phi = (qc_occ[:, None] * pc_all[None, :]).astype(f32)
    cphi = np.cos(phi, dtype=f32)
    sphi = np.sin(phi, dtype=f32)
    G_re = f32(NORM) * (cphi * Fc + sphi * Fs)
    G_im = f32(NORM) * (sphi * Fc - cphi * Fs)
    gt_re = G_re[qb_row, pbi]
    gt_im = G_im[qb_row, pbi]

    e = np.exp((q_im * q_im).astype(f32), dtype=f32)
    ang = (p_re * q_im).astype(f32)
    pr = np.clip(np.nan_to_num(f32(NORM) * e * np.cos(ang, dtype=f32)), -100.0, 100.0).astype(f32)
    pi_ = np.clip(np.nan_to_num(f32(NORM) * e * np.sin(ang, dtype=f32)), -100.0, 100.0).astype(f32)
    vr = (pr * factors_re - pi_ * factors_im).astype(f32).reshape(-1)
    vi = (pr * factors_im + pi_ * factors_re).astype(f32).reshape(-1)

    N = vr.size
    B_re = np.zeros(N, dtype=f32)
    B_im = np.zeros(N, dtype=f32)
    np.add.at(B_re, inv, vr)
    np.add.at(B_im, inv, vi)
    dr = B_re - gt_re
    di = B_im - gt_im
    loss = np.sum(dr * dr + di * di, dtype=f32)
    return np.sqrt(loss, dtype=f32)


# revision 9
# speedup vs baseline: 1.1946x; 1.0595x over previous
"""Trainium2 Bass kernel for nn_CoherentLoss (histogram_binning).

Math: the coherent-state overlap gt[n] depends on trajectory n only through its
phase-space bin (qb, pb).  With bin centers qc, pc:

  G(qb,pb) = norm * e^{i*pc*qc} * sum_m [wpsi_m * e^{-(x_m-qc)^2}] * e^{-i*pc*x_m}

The m-sum is a matmul  V[qb,m] @ [cos|sin](pc_pb * x_m)[m, pb].  The m-axis
(2401 grid points, padded to 3072 = 8 cores x 3 tiles x 128) is sharded across
the 8 NeuronCores; each core emits a partial [Q, 256] = [Fc | Fs] slab and the
host sums the 8 slabs (a 64KB reduction) and assembles the O(N) tail: binning
indices, compact-bin scatter-add, and the final sum of squares.

Device structure per core (m within a tile is affine in the partition index p:
x[p,t] = a_t + h*p, so index-structured matrices are low-rank in p x (t,bin)
and are generated on the TensorEngine from tiny coefficient rows):

  ph[p, (t,j)] = pc_j*x[p,t]/2pi          K=2 matmul  [1,p] x coeffs
  dsq[p, (t,q)] = (qc_q - x[p,t])^2       K=3 matmul  [1,p,p^2] x coeffs
  ee = Exp(-dsq)               one ACT instruction (one table load)
  red = ph - rne(ph)           VE f32->i32->f32 round-trip (range reduction)
  cs = Sin(2pi*red)            one ACT instruction; [cos|sin] via +1/4 turn
  vv = wpsi * ee               VE per-tile scalar mul
  F += vv_t^T @ cs_t           K=128 accumulating matmuls -> PSUM [Q,256]

ACT Sin is only accurate on [-pi, pi], hence the explicit range reduction.
The tail clears semaphores (no barrier) so the NEFF is re-runnable under
profiling.
"""
from contextlib import ExitStack

import numpy as np

import concourse.bass as bass
from concourse import mybir
from concourse.bass_utils import run_bass_kernel_spmd

QMIN, QMAX, QBINS = -8.0, 8.0, 128
PMIN, PMAX, PBINS = -10.0, 10.0, 128
GAMMA = 1.0
NORM = float((2.0 * GAMMA / np.pi) ** 0.25)
TWO_PI = float(2.0 * np.pi)

N_CORES = 8
f32 = np.float32

_BUILD_CACHE = {}


def _build(T, Q):
    """Build the SPMD program: T m-tiles of 128 per core, Q occupied qb rows."""
    nc = bass.Bass()
    dt = mybir.dt.float32
    CS = 2 * PBINS            # 256: [cos | sin] per tile
    WCS = T * CS              # 768
    WQ = T * Q                # 192
    WR = WCS + WQ             # 960: rhs coefficient columns
    KK = 3 + 2 * T            # lhsT rows: ones, p, sgn x T, p^2, lnw x T

    lhs_in = nc.declare_dram_parameter("lhs", [KK, 128], dt, isOutput=False)
    rhs_in = nc.declare_dram_parameter("rhs", [KK, WR], dt, isOutput=False)
    out = nc.declare_dram_parameter("out", [Q, CS], dt, isOutput=True)

    with ExitStack() as ctx:
        lhs = ctx.enter_context(nc.sbuf_tensor("lhs_s", [KK, 128], dt))
        rhs = ctx.enter_context(nc.sbuf_tensor("rhs_s", [KK, WR], dt))
        ki = ctx.enter_context(nc.sbuf_tensor("ki", [128, WCS], mybir.dt.int32))
        kf = ctx.enter_context(nc.sbuf_tensor("kf", [128, WCS], dt))
        red = ctx.enter_context(nc.sbuf_tensor("red", [128, WCS], dt))
        cs = ctx.enter_context(nc.sbuf_tensor("cs", [128, WCS], dt))
        vv = ctx.enter_context(nc.sbuf_tensor("vv", [128, WQ], dt))
        outs = ctx.enter_context(nc.sbuf_tensor("outs", [Q, CS], dt))
        scr = ctx.enter_context(nc.sbuf_tensor("scr", [1, 8], dt))
        ph = ctx.enter_context(nc.psum_tensor("ph", [128, WCS], dt))
        dq = ctx.enter_context(nc.psum_tensor("dq", [128, WQ], dt))
        ps = ctx.enter_context(nc.psum_tensor("ps", [Q, CS], dt))
        d1 = ctx.enter_context(nc.semaphore("d1"))
        p1 = ctx.enter_context(nc.semaphore("p1"))
        v1 = ctx.enter_context(nc.semaphore("v1"))
        a1 = ctx.enter_context(nc.semaphore("a1"))
        m1 = ctx.enter_context(nc.semaphore("m1"))
        c1 = ctx.enter_context(nc.semaphore("c1"))
        g1 = ctx.enter_context(nc.semaphore("g1"))
        blk = nc.Block()
        block = blk.__enter__()

        @block.sync
        def _(sync):
            sync.dma_start(out=lhs[:, :], in_=lhs_in[:, :]).then_inc(d1, 16)
            sync.dma_start(out=rhs[:, :], in_=rhs_in[:, :]).then_inc(d1, 16)

        @block.tensor
        def _(tensor):
            tensor.wait_ge(d1, 32)
            # phase/2pi: rows [ones, p, sgn x T], split 512 + 256 (PSUM bank limit)
            tensor.matmul(ph[:, 0:512], lhsT=lhs[0:2 + T, :], rhs=rhs[0:2 + T, 0:512],
                          start=True, stop=True)
            tensor.matmul(ph[:, 512:WCS], lhsT=lhs[0:2 + T, :], rhs=rhs[0:2 + T, 512:WCS],
                          start=True, stop=True).then_inc(p1, 1)
            # envelope exponent arg: all KK rows (sgn rows zero-coefficiented)
            tensor.matmul(dq[:, :], lhsT=lhs[:, :], rhs=rhs[:, WCS:WR],
                          start=True, stop=True).then_inc(p1, 1)
            # accumulating contraction over m
            tensor.wait_ge(a1, 4)
            mm = None
            for t in range(T):
                mm = tensor.matmul(ps[:, :], lhsT=vv[:, t * Q:(t + 1) * Q],
                                   rhs=cs[:, t * CS:(t + 1) * CS],
                                   start=(t == 0), stop=(t == T - 1))
            mm.then_inc(m1, 1)

        @block.vector
        def _(vector):
            vector.wait_ge(p1, 1)
            vector.tensor_copy(ki[:, :], ph[:, :])    # f32 -> i32 (round nearest)
            vector.tensor_copy(kf[:, :], ki[:, :])    # i32 -> f32
            vector.tensor_sub(red[:, :], ph[:, :], kf[:, :]).then_inc(v1, 1)

        @block.scalar
        def _(scalar):
            # warm both activation tables while DMAs are in flight
            scalar.activation(scr[:, :], scr[:, :], mybir.ActivationFunctionType.Sin)
            scalar.activation(scr[:, :], scr[:, :], mybir.ActivationFunctionType.Exp).then_inc(a1, 1)
            scalar.wait_ge(p1, 2)
            scalar.activation(vv[:, :], dq[:, :], mybir.ActivationFunctionType.Exp,
                              scale=-1.0).then_inc(a1, 1)
            scalar.wait_ge(v1, 1)
            scalar.activation(cs[:, :], red[:, :], mybir.ActivationFunctionType.Sin,
                              scale=TWO_PI).then_inc(a1, 2)
            scalar.wait_ge(m1, 1)
            scalar.copy(outs[:, :], ps[:, :]).then_inc(c1, 1)

        @block.gpsimd
        def _(gpsimd):
            gpsimd.wait_ge(c1, 1)
            gpsimd.dma_start(out=out[:, :], in_=outs[:, :]).then_inc(g1, 16)
            gpsimd.wait_ge(g1, 16)
            # restore sem/DMA state for re-execution without a full barrier:
            # every other engine's last effect was already awaited on this chain
            lo = min(s.num for s in (d1, p1, v1, a1, m1, c1, g1))
            hi = max(s.num for s in (d1, p1, v1, a1, m1, c1, g1))
            gpsimd.dma_reset(range(lo, hi + 1))
            gpsimd.sem_clear(range(lo, hi + 1))

        blk.__exit__(None, None, None)

    return nc


def _host_prep(q_re, q_im, p_re, p_im, x, psi):
    qf = q_re - p_im / f32(2.0)
    pf = f32(2.0) * q_im + p_re
    dq = f32((QMAX - QMIN) / QBINS)
    dp = f32((PMAX - PMIN) / PBINS)
    qb = np.floor((qf - f32(QMIN)) / dq)
    pb = np.floor((pf - f32(PMIN)) / dp)
    bins = (qb * PBINS + pb).astype(np.int32).reshape(-1)
    uniq, inv = np.unique(bins, return_inverse=True)
    qbi = qb.astype(np.int64).reshape(-1)
    pbi = pb.astype(np.int64).reshape(-1)
    qb_occ = np.unique(qbi)
    qb_rank = {v: i for i, v in enumerate(qb_occ)}
    qb_row = np.array([qb_rank[v] for v in qbi], dtype=np.int64)
    qc_occ = (qb_occ.astype(f32) + f32(0.5)) * dq + f32(QMIN)
    pc_all = (np.arange(PBINS, dtype=f32) + f32(0.5)) * dp + f32(PMIN)
    dx = np.diff(x)
    w = np.zeros_like(x)
    w[0] = dx[0] / 2
    w[-1] = dx[-1] / 2
    w[1:-1] = (dx[:-1] + dx[1:]) / 2
    wpsi = (w * psi).astype(f32)
    return bins, uniq, inv, qb_row, pbi, qc_occ, pc_all, wpsi


def _run_device(x, wpsi, qc_occ, pc_all, trace=False):
    M = x.shape[0]
    Qocc = qc_occ.shape[0]
    Q = max(8, int(np.ceil(Qocc / 8.0)) * 8)
    assert Q <= 128, "qb occupancy exceeds one PSUM partition tile"
    T = int(np.ceil(M / (N_CORES * 128.0)))
    Mp = N_CORES * T * 128
    CS = 2 * PBINS
    KK = 3 + 2 * T

    # grid is uniform: x[m] = x0 + m*h
    h = f32((float(x[-1]) - float(x[0])) / (M - 1))
    x0 = f32(x[0])

    wp = np.zeros(Mp, dtype=f32)
    wp[:M] = wpsi
    qc_pad = np.full(Q, 1000.0, dtype=f32)   # pad columns -> V = 0
    qc_pad[:Qocc] = qc_occ

    ws = wp.reshape(N_CORES, T, 128)

    key = (T, Q)
    if key not in _BUILD_CACHE:
        _BUILD_CACHE[key] = _build(T, Q)
    nc = _BUILD_CACHE[key]

    p_idx = np.arange(128, dtype=f32)
    r1 = (pc_all * h / f32(TWO_PI)).astype(f32)

    in_maps = []
    for c in range(N_CORES):
        # x[p, t] = a_t + h*p for this core's tiles
        a_t = (x0 + h * (np.arange(T, dtype=f32) + f32(c * T)) * f32(128.0)).astype(f32)
        wct = ws[c]                                   # [T, 128]
        # lhsT rows: ones, p, sgn x T (0.5 where wpsi<0), p^2, lnw x T (clamped)
        lhs = np.zeros((KK, 128), dtype=f32)
        lhs[0] = 1.0
        lhs[1] = p_idx
        lhs[2 + T] = p_idx * p_idx
        with np.errstate(divide="ignore"):
            lnw = np.log(np.abs(wct)).astype(f32)
        lnw = np.maximum(lnw, f32(-100.0))
        for t in range(T):
            lhs[2 + t] = f32(0.5) * (wct[t] < 0)
            lhs[3 + T + t] = lnw[t]
        rhs = np.zeros((KK, T * CS + T * Q), dtype=f32)
        for t in range(T):
            base = t * CS
            r0 = (pc_all * a_t[t] / f32(TWO_PI)).astype(f32)
            rhs[0, base:base + PBINS] = r0 + f32(0.25)   # cos args (quarter turn)
            rhs[0, base + PBINS:base + CS] = r0          # sin args
            rhs[1, base:base + CS] = np.concatenate([r1, r1])
            rhs[2 + t, base:base + CS] = 1.0             # sign half-turn for tile t
            qbase = T * CS + t * Q
            dqa = (qc_pad - a_t[t]).astype(f32)
            rhs[0, qbase:qbase + Q] = dqa * dqa
            rhs[1, qbase:qbase + Q] = f32(-2.0) * h * dqa
            rhs[2 + T, qbase:qbase + Q] = h * h
            rhs[3 + T + t, qbase:qbase + Q] = -1.0       # -ln|wpsi| for tile t
        in_maps.append({"lhs": lhs, "rhs": rhs})

    res = run_bass_kernel_spmd(nc, in_maps, core_ids=list(range(N_CORES)), trace=trace)
    F = np.zeros((Q, CS), dtype=np.float64)
    for c in range(N_CORES):
        F += res.results[c]["out"]
    F = F.astype(f32)
    return F[:Qocc, :PBINS], F[:Qocc, PBINS:], res


def kernel(factors_re, factors_im, q_re, q_im, p_re, p_im, x, psi):
    factors_re = np.asarray(factors_re, dtype=f32)
    factors_im = np.asarray(factors_im, dtype=f32)
    q_re = np.asarray(q_re, dtype=f32)
    q_im = np.asarray(q_im, dtype=f32)
    p_re = np.asarray(p_re, dtype=f32)
    p_im = np.asarray(p_im, dtype=f32)
    x = np.asarray(x, dtype=f32)
    psi = np.asarray(psi, dtype=f32)

    bins, uniq, inv, qb_row, pbi, qc_occ, pc_all, wpsi = _host_prep(
        q_re, q_im, p_re, p_im, x, psi
    )
    Fc, Fs, _ = _run_device(x, wpsi, qc_occ, pc_all)

    # ---- host tail: phase correction, gather, scatter-add, loss ----
    phi = (qc_occ[:, None] * pc_all[None, :]).astype(f32)
    cphi = np.cos(phi, dtype=f32)
    sphi = np.sin(phi, dtype=f32)
    G_re = f32(NORM) * (cphi * Fc + sphi * Fs)
    G_im = f32(NORM) * (sphi * Fc - cphi * Fs)
    gt_re = G_re[qb_row, pbi]
    gt_im = G_im[qb_row, pbi]

    e = np.exp((q_im * q_im).astype(f32), dtype=f32)
    ang = (p_re * q_im).astype(f32)
    pr = np.clip(np.nan_to_num(f32(NORM) * e * np.cos(ang, dtype=f32)), -100.0, 100.0).astype(f32)
    pi_ = np.clip(np.nan_to_num(f32(NORM) * e * np.sin(ang, dtype=f32)), -100.0, 100.0).astype(f32)
    vr = (pr * factors_re - pi_ * factors_im).astype(f32).reshape(-1)
    vi = (pr * factors_im + pi_ * factors_re).astype(f32).reshape(-1)

    N = vr.size
    B_re = np.zeros(N, dtype=f32)
    B_im = np.zeros(N, dtype=f32)
    np.add.at(B_re, inv, vr)
    np.add.at(B_im, inv, vi)
    dr = B_re - gt_re
    di = B_im - gt_im
    loss = np.sum(dr * dr + di * di, dtype=f32)
    return np.sqrt(loss, dtype=f32)


# revision 10
# speedup vs baseline: 1.2587x; 1.0537x over previous
"""Trainium2 Bass kernel for nn_CoherentLoss (histogram_binning).

Math: the coherent-state overlap gt[n] depends on trajectory n only through its
phase-space bin (qb, pb).  With bin centers qc, pc:

  G(qb,pb) = norm * e^{i*pc*qc} * sum_m [wpsi_m * e^{-(x_m-qc)^2}] * e^{-i*pc*x_m}

The m-sum is a matmul  V[qb,m] @ [cos|sin](pc_pb * x_m)[m, pb].  The m-axis
(2401 grid points, padded to 3072 = 8 cores x 3 tiles x 128) is sharded across
the 8 NeuronCores; each core emits a partial [Q, 256] = [Fc | Fs] slab and the
host sums the 8 slabs (a 64KB reduction) and assembles the O(N) tail: binning
indices, compact-bin scatter-add, and the final sum of squares.

Device structure per core (m within a tile is affine in the partition index p:
x[p,t] = a_t + h*p, so index-structured matrices are low-rank in p x (t,bin)
and are generated on the TensorEngine from tiny coefficient rows):

  ph[p, (t,j)] = pc_j*x[p,t]/2pi          K=2 matmul  [1,p] x coeffs
  dsq[p, (t,q)] = (qc_q - x[p,t])^2       K=3 matmul  [1,p,p^2] x coeffs
  ee = Exp(-dsq)               one ACT instruction (one table load)
  red = ph - rne(ph)           VE f32->i32->f32 round-trip (range reduction)
  cs = Sin(2pi*red)            one ACT instruction; [cos|sin] via +1/4 turn
  vv = wpsi * ee               VE per-tile scalar mul
  F += vv_t^T @ cs_t           K=128 accumulating matmuls -> PSUM [Q,256]

ACT Sin is only accurate on [-pi, pi], hence the explicit range reduction.
The tail clears semaphores (no barrier) so the NEFF is re-runnable under
profiling.
"""
from contextlib import ExitStack

import numpy as np

import concourse.bass as bass
from concourse import mybir
from concourse.bass_utils import run_bass_kernel_spmd

QMIN, QMAX, QBINS = -8.0, 8.0, 128
PMIN, PMAX, PBINS = -10.0, 10.0, 128
GAMMA = 1.0
NORM = float((2.0 * GAMMA / np.pi) ** 0.25)
TWO_PI = float(2.0 * np.pi)

N_CORES = 8
f32 = np.float32

_BUILD_CACHE = {}


def _build(T, Q):
    """Build the SPMD program: T m-tiles of 128 per core, Q occupied qb rows."""
    nc = bass.Bass()
    dt = mybir.dt.float32
    CS = 2 * PBINS            # 256: [cos | sin] per tile
    WCS = T * CS              # 768
    WQ = T * Q                # 192
    WR = WCS + WQ             # 960: rhs coefficient columns
    KK = 3 + 2 * T            # lhsT rows: ones, p, sgn x T, p^2, lnw x T

    lhs_in = nc.declare_dram_parameter("lhs", [KK, 128], dt, isOutput=False)
    rhs_in = nc.declare_dram_parameter("rhs", [KK, WR], dt, isOutput=False)
    out = nc.declare_dram_parameter("out", [Q, CS], dt, isOutput=True)

    with ExitStack() as ctx:
        lhs = ctx.enter_context(nc.sbuf_tensor("lhs_s", [KK, 128], dt))
        rhs = ctx.enter_context(nc.sbuf_tensor("rhs_s", [KK, WR], dt))
        ki = ctx.enter_context(nc.sbuf_tensor("ki", [128, WCS], mybir.dt.int32))
        kf = ctx.enter_context(nc.sbuf_tensor("kf", [128, WCS], dt))
        red = ctx.enter_context(nc.sbuf_tensor("red", [128, WCS], dt))
        cs = ctx.enter_context(nc.sbuf_tensor("cs", [128, WCS], dt))
        vv = ctx.enter_context(nc.sbuf_tensor("vv", [128, WQ], dt))
        outs = ctx.enter_context(nc.sbuf_tensor("outs", [Q, CS], dt))
        scr = ctx.enter_context(nc.sbuf_tensor("scr", [1, 8], dt))
        ph = ctx.enter_context(nc.psum_tensor("ph", [128, WCS], dt))
        dq = ctx.enter_context(nc.psum_tensor("dq", [128, WQ], dt))
        ps = ctx.enter_context(nc.psum_tensor("ps", [Q, CS], dt))
        d1 = ctx.enter_context(nc.semaphore("d1"))
        p1 = ctx.enter_context(nc.semaphore("p1"))
        v1 = ctx.enter_context(nc.semaphore("v1"))
        a1 = ctx.enter_context(nc.semaphore("a1"))
        m1 = ctx.enter_context(nc.semaphore("m1"))
        c1 = ctx.enter_context(nc.semaphore("c1"))
        g1 = ctx.enter_context(nc.semaphore("g1"))
        blk = nc.Block(no_gpsimd_drain=True)
        block = blk.__enter__()

        @block.sync
        def _(sync):
            sync.dma_start(out=lhs[:, :], in_=lhs_in[:, :]).then_inc(d1, 16)
            sync.dma_start(out=rhs[:, :], in_=rhs_in[:, :]).then_inc(d1, 16)

        @block.tensor
        def _(tensor):
            tensor.wait_ge(d1, 32)
            # phase/2pi: rows [ones, p, sgn x T], split 512 + 256 (PSUM bank limit)
            tensor.matmul(ph[:, 0:512], lhsT=lhs[0:2 + T, :], rhs=rhs[0:2 + T, 0:512],
                          start=True, stop=True).then_inc(p1, 1)
            tensor.matmul(ph[:, 512:WCS], lhsT=lhs[0:2 + T, :], rhs=rhs[0:2 + T, 512:WCS],
                          start=True, stop=True).then_inc(p1, 1)
            # envelope exponent arg: all KK rows (sgn rows zero-coefficiented)
            tensor.matmul(dq[:, :], lhsT=lhs[:, :], rhs=rhs[:, WCS:WR],
                          start=True, stop=True).then_inc(p1, 1)
            # accumulating contraction over m
            tensor.wait_ge(a1, 4)
            mm = None
            for t in range(T):
                mm = tensor.matmul(ps[:, :], lhsT=vv[:, t * Q:(t + 1) * Q],
                                   rhs=cs[:, t * CS:(t + 1) * CS],
                                   start=(t == 0), stop=(t == T - 1))
            mm.then_inc(m1, 1)

        @block.vector
        def _(vector):
            vector.wait_ge(p1, 1)
            vector.tensor_copy(ki[:, 0:512], ph[:, 0:512])      # f32 -> i32 (rne)
            vector.wait_ge(p1, 2)
            vector.tensor_copy(ki[:, 512:WCS], ph[:, 512:WCS])
            vector.tensor_copy(kf[:, :], ki[:, :])              # i32 -> f32
            vector.tensor_sub(red[:, :], ph[:, :], kf[:, :]).then_inc(v1, 1)

        @block.scalar
        def _(scalar):
            # warm the Exp table while DMAs are in flight
            scalar.activation(scr[:, :], scr[:, :], mybir.ActivationFunctionType.Exp).then_inc(a1, 1)
            scalar.wait_ge(p1, 3)
            scalar.activation(vv[:, :], dq[:, :], mybir.ActivationFunctionType.Exp,
                              scale=-1.0).then_inc(a1, 1)
            # warm the Sin table while the VE range reduction runs
            scalar.activation(scr[:, :], scr[:, :], mybir.ActivationFunctionType.Sin)
            scalar.wait_ge(v1, 1)
            scalar.activation(cs[:, :], red[:, :], mybir.ActivationFunctionType.Sin,
                              scale=TWO_PI).then_inc(a1, 2)
            scalar.wait_ge(m1, 1)
            scalar.copy(outs[:, :], ps[:, :]).then_inc(c1, 1)

        @block.gpsimd
        def _(gpsimd):
            gpsimd.wait_ge(c1, 1)
            gpsimd.dma_start(out=out[:, :], in_=outs[:, :]).then_inc(g1, 16)
            gpsimd.wait_ge(g1, 16)
            # restore sem/DMA state for re-execution without a full barrier:
            # every other engine's last effect was already awaited on this chain
            lo = min(s.num for s in (d1, p1, v1, a1, m1, c1, g1))
            hi = max(s.num for s in (d1, p1, v1, a1, m1, c1, g1))
            gpsimd.dma_reset(range(lo, hi + 1))
            gpsimd.sem_clear(range(lo, hi + 1))

        blk.__exit__(None, None, None)

    return nc


def _host_prep(q_re, q_im, p_re, p_im, x, psi):
    qf = q_re - p_im / f32(2.0)
    pf = f32(2.0) * q_im + p_re
    dq = f32((QMAX - QMIN) / QBINS)
    dp = f32((PMAX - PMIN) / PBINS)
    qb = np.floor((qf - f32(QMIN)) / dq)
    pb = np.floor((pf - f32(PMIN)) / dp)
    bins = (qb * PBINS + pb).astype(np.int32).reshape(-1)
    uniq, inv = np.unique(bins, return_inverse=True)
    qbi = qb.astype(np.int64).reshape(-1)
    pbi = pb.astype(np.int64).reshape(-1)
    qb_occ = np.unique(qbi)
    qb_rank = {v: i for i, v in enumerate(qb_occ)}
    qb_row = np.array([qb_rank[v] for v in qbi], dtype=np.int64)
    qc_occ = (qb_occ.astype(f32) + f32(0.5)) * dq + f32(QMIN)
    pc_all = (np.arange(PBINS, dtype=f32) + f32(0.5)) * dp + f32(PMIN)
    dx = np.diff(x)
    w = np.zeros_like(x)
    w[0] = dx[0] / 2
    w[-1] = dx[-1] / 2
    w[1:-1] = (dx[:-1] + dx[1:]) / 2
    wpsi = (w * psi).astype(f32)
    return bins, uniq, inv, qb_row, pbi, qc_occ, pc_all, wpsi


def _run_device(x, wpsi, qc_occ, pc_all, trace=False):
    M = x.shape[0]
    Qocc = qc_occ.shape[0]
    Q = max(8, int(np.ceil(Qocc / 8.0)) * 8)
    assert Q <= 128, "qb occupancy exceeds one PSUM partition tile"
    T = int(np.ceil(M / (N_CORES * 128.0)))
    Mp = N_CORES * T * 128
    CS = 2 * PBINS
    KK = 3 + 2 * T

    # grid is uniform: x[m] = x0 + m*h
    h = f32((float(x[-1]) - float(x[0])) / (M - 1))
    x0 = f32(x[0])

    wp = np.zeros(Mp, dtype=f32)
    wp[:M] = wpsi
    qc_pad = np.full(Q, 1000.0, dtype=f32)   # pad columns -> V = 0
    qc_pad[:Qocc] = qc_occ

    ws = wp.reshape(N_CORES, T, 128)

    key = (T, Q)
    if key not in _BUILD_CACHE:
        _BUILD_CACHE[key] = _build(T, Q)
    nc = _BUILD_CACHE[key]

    p_idx = np.arange(128, dtype=f32)
    r1 = (pc_all * h / f32(TWO_PI)).astype(f32)

    in_maps = []
    for c in range(N_CORES):
        # x[p, t] = a_t + h*p for this core's tiles
        a_t = (x0 + h * (np.arange(T, dtype=f32) + f32(c * T)) * f32(128.0)).astype(f32)
        wct = ws[c]                                   # [T, 128]
        # lhsT rows: ones, p, sgn x T (0.5 where wpsi<0), p^2, lnw x T (clamped)
        lhs = np.zeros((KK, 128), dtype=f32)
        lhs[0] = 1.0
        lhs[1] = p_idx
        lhs[2 + T] = p_idx * p_idx
        with np.errstate(divide="ignore"):
            lnw = np.log(np.abs(wct)).astype(f32)
        lnw = np.maximum(lnw, f32(-100.0))
        for t in range(T):
            lhs[2 + t] = f32(0.5) * (wct[t] < 0)
            lhs[3 + T + t] = lnw[t]
        rhs = np.zeros((KK, T * CS + T * Q), dtype=f32)
        for t in range(T):
            base = t * CS
            r0 = (pc_all * a_t[t] / f32(TWO_PI)).astype(f32)
            rhs[0, base:base + PBINS] = r0 + f32(0.25)   # cos args (quarter turn)
            rhs[0, base + PBINS:base + CS] = r0          # sin args
            rhs[1, base:base + CS] = np.concatenate([r1, r1])
            rhs[2 + t, base:base + CS] = 1.0             # sign half-turn for tile t
            qbase = T * CS + t * Q
            dqa = (qc_pad - a_t[t]).astype(f32)
            rhs[0, qbase:qbase + Q] = dqa * dqa
            rhs[1, qbase:qbase + Q] = f32(-2.0) * h * dqa
            rhs[2 + T, qbase:qbase + Q] = h * h
            rhs[3 + T + t, qbase:qbase + Q] = -1.0       # -ln|wpsi| for tile t
        in_maps.append({"lhs": lhs, "rhs": rhs})

    res = run_bass_kernel_spmd(nc, in_maps, core_ids=list(range(N_CORES)), trace=trace)
    F = np.zeros((Q, CS), dtype=np.float64)
    for c in range(N_CORES):
        F += res.results[c]["out"]
    F = F.astype(f32)
    return F[:Qocc, :PBINS], F[:Qocc, PBINS:], res


def kernel(factors_re, factors_im, q_re, q_im, p_re, p_im, x, psi):
    factors_re = np.asarray(factors_re, dtype=f32)
    factors_im = np.asarray(factors_im, dtype=f32)
    q_re = np.asarray(q_re, dtype=f32)
    q_im = np.asarray(q_im, dtype=f32)
    p_re = np.asarray(p_re, dtype=f32)
    p_im = np.asarray(p_im, dtype=f32)
    x = np.asarray(x, dtype=f32)
    psi = np.asarray(psi, dtype=f32)

    bins, uniq, inv, qb_row, pbi, qc_occ, pc_all, wpsi = _host_prep(
        q_re, q_im, p_re, p_im, x, psi
    )
    Fc, Fs, _ = _run_device(x, wpsi, qc_occ, pc_all)

    # ---- host tail: phase correction, gather, scatter-add, loss ----
    phi = (qc_occ[:, None] * pc_all[None, :]).astype(f32)
    cphi = np.cos(phi, dtype=f32)
    sphi = np.sin(phi, dtype=f32)
    G_re = f32(NORM) * (cphi * Fc + sphi * Fs)
    G_im = f32(NORM) * (sphi * Fc - cphi * Fs)
    gt_re = G_re[qb_row, pbi]
    gt_im = G_im[qb_row, pbi]

    e = np.exp((q_im * q_im).astype(f32), dtype=f32)
    ang = (p_re * q_im).astype(f32)
    pr = np.clip(np.nan_to_num(f32(NORM) * e * np.cos(ang, dtype=f32)), -100.0, 100.0).astype(f32)
    pi_ = np.clip(np.nan_to_num(f32(NORM) * e * np.sin(ang, dtype=f32)), -100.0, 100.0).astype(f32)
    vr = (pr * factors_re - pi_ * factors_im).astype(f32).reshape(-1)
    vi = (pr * factors_im + pi_ * factors_re).astype(f32).reshape(-1)

    N = vr.size
    B_re = np.zeros(N, dtype=f32)
    B_im = np.zeros(N, dtype=f32)
    np.add.at(B_re, inv, vr)
    np.add.at(B_im, inv, vi)
    dr = B_re - gt_re
    di = B_im - gt_im
    loss = np.sum(dr * dr + di * di, dtype=f32)
    return np.sqrt(loss, dtype=f32)
